# revision 1
# baseline (speedup 1.0000x reference)
"""Trainium2 Bass kernel for a dense transformer encoder layer.

Shapes (from the problem): B=4, S=2048, D=512, H=8 heads (HD=64), FFN F=2048.

Sharding (8 NeuronCores, no collectives):
  core c handles batch b = c//2 and query-half  half = c%2 (1024 query tokens).
  K/V are computed for the full 2048-token sequence of the batch on both cores
  of a pair (duplicated K/V projection ~ +8% FLOPs, zero communication).

Device layout: all activations are kept TRANSPOSED ([d, token]) so every
matmul contraction has d on the PSUM partition axis and tokens are the moving
operand; LayerNorm statistics are computed with ones-vector matmuls on the PE
and applied via rank-1 broadcast matmuls. Softmax normalization uses an extra
ones-column appended to V (denominator rides along in the AV matmul).
The host transposes x / un-transposes the output (free in numpy).

All matmul operands are bf16 (same 1 column/cycle PE speed as float32r but
half the DMA traffic and SBUF footprint); PSUM accumulation, the residual
spine (x, res1, res2) and LayerNorm statistics stay fp32. FFN weights are
prefetched during attention; LayerNorm stats are interleaved into the
producing matmul loops and the apply runs in token halves so the next
phase's matmuls start early.
"""

import functools
import numpy as np
from contextlib import ExitStack

import ml_dtypes

import concourse.bass as bass
import concourse.tile as tile
import concourse.mybir as mybir
from concourse import bacc
from concourse.bass import ts
from concourse.vector_clock import ScopedClock

B, S, D, H, F = 4, 2048, 512, 8, 2048
HD = D // H           # 64
P = 128
DC = D // P           # 4  d chunks
FC = F // P           # 16 ffn chunks
SC = S // P           # 16 seq chunks
TOK = S // 2          # 1024 query tokens per core
NSL = TOK // 512      # 2 moving slices of 512
EPS = 1e-5
VW = HD + 1           # 65: V columns per head incl. ones column

f32 = mybir.dt.float32
f32r = mybir.dt.float32r
bf16 = mybir.dt.bfloat16
AF = mybir.ActivationFunctionType
ALU = mybir.AluOpType


class _TC(tile.TileContext):
    """TileContext whose tail drain splits sem waits one-per-drain: the
    walrus build in this container rejects >1 sync wait on an SP TPB_CTRL."""

    def _drain_and_barrier(self, tick_clock, wait_clock):
        nc = self.nc
        drain_inst = nc.sync.drain()
        wait_clock.add_sem_waits(
            drain_inst.ins, ScopedClock({None: tick_clock.global_clock})
        )
        si = drain_inst.ins.sync_info
        waits = list(si.on_wait) if si and si.on_wait else []
        MAXW = 1
        if len(waits) > MAXW:
            si.on_wait = waits[:MAXW]
            for i in range(MAXW, len(waits), MAXW):
                extra = nc.sync.drain()
                extra.ins.sync_info = mybir.SyncInfo(
                    on_wait=waits[i : i + MAXW], on_update=[]
                )
        nc.all_engine_barrier()
        popped = nc._tile_sem_poison_stack.pop()
        assert popped is self._sem_poison
        nc.clear_and_free_semaphores(list(self.sems.allocated().values()))
        nc.all_engine_barrier()


def _r(ap):
    return ap.bitcast(f32r)


def _bcast_ap(row_ap, nparts):
    """AP reading the single-partition row `row_ap` broadcast to nparts."""
    return bass.AP(
        tensor=row_ap.tensor,
        offset=row_ap.offset,
        ap=[[0, nparts]] + [list(d) for d in row_ap.ap[1:]],
    )


def _ln_stats_seg(nc, sum_ps, sq_ps, src, sq_tile, ones128, c, s5):
    """Emit sum/sqsum ones-matmuls for d-chunk c, token slice s5."""
    nc.scalar.activation(sq_tile, src[:, c, s5], AF.Square)
    nc.tensor.matmul(
        sum_ps[:, s5], lhsT=ones128, rhs=src[:, c, s5],
        start=(c == 0), stop=(c == DC - 1), skip_group_check=True,
    )
    nc.tensor.matmul(
        sq_ps[:, s5], lhsT=ones128, rhs=sq_tile,
        start=(c == 0), stop=(c == DC - 1), skip_group_check=True,
    )


def _ln_alloc(nc, stat_pool, tagp):
    """Allocate LN chain tiles + broadcast planes."""
    return {
        "mu": stat_pool.tile([1, TOK], f32, name=f"{tagp}_mu", tag=f"{tagp}_mu"),
        "tB": stat_pool.tile([1, TOK], f32, name=f"{tagp}_tB", tag=f"{tagp}_tB"),
        "var": stat_pool.tile([1, TOK], f32, name=f"{tagp}_var", tag=f"{tagp}_var"),
        "a": stat_pool.tile([1, TOK], f32, name=f"{tagp}_a", tag=f"{tagp}_a"),
        "b": stat_pool.tile([1, TOK], f32, name=f"{tagp}_b", tag=f"{tagp}_b"),
        "abf": stat_pool.tile([1, TOK], bf16, name=f"{tagp}_abf",
                              tag=f"{tagp}_abf"),
        "bbf": stat_pool.tile([1, TOK], bf16, name=f"{tagp}_bbf",
                              tag=f"{tagp}_bbf"),
        "bcA": stat_pool.tile([P, TOK], bf16, name=f"{tagp}_bcA",
                              tag=f"{tagp}_bcA"),
        "bcB": stat_pool.tile([P, TOK], bf16, name=f"{tagp}_bcB",
                              tag=f"{tagp}_bcB"),
    }


def _ln_chain(nc, t, sum_ps, sq_ps, eps_sb, segs):
    """Per token segment (offset, width): a = 1/sqrt(var+eps), b = -mu*a,
    then broadcast bf16 copies of (a, b) to all 128 partitions (Pool)."""
    for off, w in segs:
        s5 = slice(off, off + w)
        # mu^2 on ACT in parallel with -mu on DVE
        nc.scalar.activation(t["tB"][:, s5], sum_ps[:, s5], AF.Square,
                             scale=1.0 / D)
        nc.vector.tensor_scalar_mul(t["mu"][:, s5], sum_ps[:, s5], -1.0 / D)
        nc.vector.scalar_tensor_tensor(
            out=t["var"][:, s5], in0=sq_ps[:, s5], scalar=1.0 / D,
            in1=t["tB"][:, s5], op0=ALU.mult, op1=ALU.subtract,
        )
        nc.scalar.activation(t["tB"][:, s5], t["var"][:, s5], AF.Sqrt,
                             bias=eps_sb, scale=1.0)
        nc.vector.reciprocal(t["a"][:, s5], t["tB"][:, s5])
        nc.vector.tensor_mul(t["b"][:, s5], t["mu"][:, s5], t["a"][:, s5])
        # bf16 casts on the (idle) ACT engine keep the combine TTs in DVE 2x
        # mode without serializing behind Pool's broadcasts
        nc.scalar.copy(t["abf"][:, s5], t["a"][:, s5])
        nc.scalar.copy(t["bbf"][:, s5], t["b"][:, s5])
        nc.gpsimd.partition_broadcast(t["bcA"][:, s5], t["abf"][:, s5])
        nc.gpsimd.partition_broadcast(t["bcB"][:, s5], t["bbf"][:, s5])


def _ln_combine(nc, t, work_pool, src, dst, post, tagp, segs):
    """dst = src*a + b per (segment, chunk) as two 2x-mode TensorTensor ops on
    all-bf16 SBUF operands; post(c, s5) (optional) consumes dst[:, c, s5]."""
    for off, w_ in segs:
        s5 = slice(off, off + w_)
        for c in range(DC):
            v = work_pool.tile([P, w_], bf16, name=f"{tagp}_v",
                               tag=f"{tagp}_v", bufs=2)
            nc.vector.tensor_mul(v, src[:, c, s5], t["bcA"][:, s5])
            nc.vector.tensor_tensor(dst[:, c, s5], v, t["bcB"][:, s5],
                                    op=ALU.add)
            if post is not None:
                post(c, s5)


@functools.lru_cache(maxsize=1)
def _build_program():
    nc = bacc.Bacc()

    def dp(name, shape, out=False, dt=f32):
        return nc.declare_dram_parameter(name, list(shape), dt, isOutput=out)

    xqT_d = dp("xqT", [P, DC, TOK], dt=bf16)
    xbT_d = dp("xbT", [P, DC, S], dt=bf16)
    wqkv_d = dp("wqkv", [P, DC, 3 * D], dt=bf16)
    wo_d = dp("wo", [P, DC, D], dt=bf16)
    w1_d = dp("w1", [P, DC, F], dt=bf16)
    w2_d = dp("w2", [P, FC, D], dt=bf16)
    bqkvT_d = dp("bqkvT", [P, 12])
    boT_d = dp("boT", [P, DC])
    b1pT_d = dp("b1pT", [P, FC])      # b1 + W1^T beta1
    b2pT_d = dp("b2pT", [P, DC])      # b2 + beta1
    bvrow_d = dp("bvrow", [1, D])
    ident_d = dp("ident", [P, P], dt=bf16)
    ones_col_d = dp("ones_col", [P, 1], dt=bf16)
    g1T_d = dp("g1T", [P, DC])
    g2T_d = dp("g2T", [P, DC])
    beta2T_d = dp("beta2T", [P, DC])
    outT_d = dp("outT", [P, DC, TOK], out=True, dt=bf16)

    with _TC(nc) as tc, ExitStack() as top:
        top.enter_context(
            nc.allow_low_precision(reason="bf16 matmul pipeline by design")
        )
        persist = top.enter_context(tc.tile_pool(name="persist", bufs=1))
        bqkvT_sb = persist.tile([P, 12], f32)
        boT_sb = persist.tile([P, DC], f32)
        b1pT_sb = persist.tile([P, FC], f32)
        b2pT_sb = persist.tile([P, DC], f32)
        g1T_sb = persist.tile([P, DC], f32)
        g2T_sb = persist.tile([P, DC], f32)
        beta2T_sb = persist.tile([P, DC], f32)
        bvb_sb = persist.tile([P, D], f32)
        ones128 = persist.tile([P, 1], bf16)
        eps_sb = persist.tile([1, 1], f32)
        nc.vector.memset(eps_sb, EPS)
        ln1T_sb = persist.tile([P, DC, TOK], bf16)
        ident_sb = persist.tile([P, P], bf16)

        def _persist_loads():
            # issued AFTER the phase-1-critical DMAs: none of these are
            # needed in the first ~10us
            nc.sync.dma_start(out=ident_sb, in_=ident_d[:])
            nc.sync.dma_start(out=bqkvT_sb, in_=bqkvT_d[:])
            nc.sync.dma_start(out=boT_sb, in_=boT_d[:])
            nc.sync.dma_start(out=b1pT_sb, in_=b1pT_d[:])
            nc.sync.dma_start(out=b2pT_sb, in_=b2pT_d[:])
            nc.sync.dma_start(out=g1T_sb, in_=g1T_d[:])
            nc.sync.dma_start(out=g2T_sb, in_=g2T_d[:])
            nc.sync.dma_start(out=beta2T_sb, in_=beta2T_d[:])
            nc.gpsimd.dma_start(out=bvb_sb, in_=_bcast_ap(bvrow_d[:], P))
            nc.sync.dma_start(out=ones128, in_=ones_col_d[:])

        # FFN weights: pool open for the whole kernel, DMAs queued behind the
        # phase-1 loads so they stream in during attention.
        wP = top.enter_context(tc.tile_pool(name="wP", bufs=1))
        w1_sb = wP.tile([P, DC, F], bf16)
        w2_sb = wP.tile([P, FC, D], bf16)

        # survives past the attention scope: LN1 inputs for the apply stage
        mid = top.enter_context(tc.tile_pool(name="mid", bufs=1))
        res1_sb = mid.tile([P, DC, TOK], bf16)
        ln1t = _ln_alloc(nc, mid, "ln1")

        with ExitStack() as attn_scope:
            # tiles that survive until LN1 stats
            attnS = attn_scope.enter_context(tc.tile_pool(name="attnS", bufs=1))
            wo_sb = attnS.tile([P, DC, D], bf16)
            ctxT_sb = attnS.tile([P, DC, TOK], bf16)

            with ExitStack() as big_scope:
                attnB = big_scope.enter_context(tc.tile_pool(name="attnB", bufs=1))
                QT_sb = attnB.tile([P, DC, TOK], bf16)
                KT_sb = attnB.tile([P, DC, S], bf16)
                V_sb = attnB.tile([P, SC, H * VW], bf16)
                vcols = V_sb.rearrange("p k (h e) -> p k h e", e=VW)
                nc.gpsimd.memset(vcols[:, :, :, HD:VW], 1.0)

                # ------------- phase 1: Q + K(m=0) projections -------------
                # V and K(m=1..3) are deferred into the attention loop as PE
                # filler work: attention is exp(ACT)-bound with ~25% PE slack.
                wqkv_sb = attnB.tile([P, DC, 3 * D], bf16)
                xbT_sb = attnB.tile([P, DC, S], bf16)
                xqT_sb = attnB.tile([P, DC, TOK], bf16)
                # DMA queue order = need order; one strided DMA per tensor
                # region keeps the HWDGE dispatch count (625ns each) low.
                # Only Q(m=0) + K(m=0, quarter 0) are needed before attention
                # starts; everything else streams in under the exp-bound
                # attention phase.
                nc.sync.dma_start(out=xqT_sb[:, 0, :], in_=xqT_d[:, 0, :])
                nc.sync.dma_start(out=wqkv_sb[:, :, 0:P], in_=wqkv_d[:, :, 0:P])
                for c in range(1, DC):
                    nc.sync.dma_start(out=xqT_sb[:, c, :], in_=xqT_d[:, c, :])
                nc.sync.dma_start(out=wqkv_sb[:, :, D : D + P],
                                  in_=wqkv_d[:, :, D : D + P])
                nc.sync.dma_start(out=xbT_sb[:, :, 0:512],
                                  in_=xbT_d[:, :, 0:512])
                _persist_loads()
                nc.sync.dma_start(out=wqkv_sb[:, :, 2 * D : 3 * D],
                                  in_=wqkv_d[:, :, 2 * D : 3 * D])
                for q in range(1, 4):
                    nc.sync.dma_start(out=xbT_sb[:, :, ts(q, 512)],
                                      in_=xbT_d[:, :, ts(q, 512)])
                nc.sync.dma_start(out=wqkv_sb[:, :, P:D],
                                  in_=wqkv_d[:, :, P:D])
                nc.sync.dma_start(out=wqkv_sb[:, :, D + P : 2 * D],
                                  in_=wqkv_d[:, :, D + P : 2 * D])
                nc.sync.dma_start(out=wo_sb, in_=wo_d[:])
                nc.sync.dma_start(out=w1_sb, in_=w1_d[:])
                for c in range(0, FC, 8):
                    nc.sync.dma_start(
                        out=w2_sb[:, c : c + 8, :], in_=w2_d[:, c : c + 8, :])

                with tc.tile_pool(name="q_ps", bufs=1, space="PSUM") as psQ:
                    # Q^T m=0 (heads 0-1)
                    q_ps = psQ.tile([P, TOK], f32, name="q_ps", tag="q")
                    for c in range(DC):
                        for sl in range(NSL):
                            nc.tensor.matmul(
                                q_ps[:, ts(sl, 512)],
                                lhsT=wqkv_sb[:, c, 0:P],
                                rhs=xqT_sb[:, c, ts(sl, 512)],
                                start=(c == 0), stop=(c == DC - 1),
                            )
                    nc.vector.tensor_scalar_add(
                        QT_sb[:, 0, :], q_ps, bqkvT_sb[:, 0:1]
                    )
                    # K^T m=0 quarter 0 (first 4 key chunks of heads 0-1)
                    k_ps = psQ.tile([P, 512], f32, name="k_ps", tag="k")
                    for c in range(DC):
                        nc.tensor.matmul(
                            k_ps,
                            lhsT=wqkv_sb[:, c, D : D + P],
                            rhs=xbT_sb[:, c, 0:512],
                            start=(c == 0), stop=(c == DC - 1),
                        )
                    nc.vector.tensor_scalar_add(
                        KT_sb[:, 0, 0:512], k_ps, bqkvT_sb[:, 4:5],
                    )

                # ------------- phase 2: attention + deferred K/V -------------
                bvb_h = bvb_sb.rearrange("p (h e) -> p h e", e=HD)
                kv_scope = ExitStack()
                with tc.tile_pool(name="sc_ps", bufs=2, space="PSUM") as psSc, \
                     tc.tile_pool(name="expP", bufs=6) as expP, \
                     tc.tile_pool(name="bcP", bufs=2) as bcP:
                    psKV = kv_scope.enter_context(
                        tc.tile_pool(name="kv_ps", bufs=2, space="PSUM"))
                    psCtx = kv_scope.enter_context(
                        tc.tile_pool(name="ctx_ps", bufs=1, space="PSUM"))

                    def emit_v(kc):
                        v_ps = psKV.tile([P, D], f32, name="v_ps", tag="vk")
                        for c in range(DC):
                            nc.tensor.matmul(
                                v_ps,
                                lhsT=xbT_sb[:, c, ts(kc, P)],
                                rhs=wqkv_sb[:, c, 2 * D : 3 * D],
                                start=(c == 0), stop=(c == DC - 1),
                                skip_group_check=True,
                            )
                        nc.vector.tensor_tensor(
                            vcols[:, kc, :, 0:HD],
                            v_ps.rearrange("p (h e) -> p h e", e=HD),
                            bvb_h, op=ALU.add,
                        )

                    def emit_k(m, q):
                        k_ps = psKV.tile([P, 512], f32, name="k_ps2", tag="vk")
                        for c in range(DC):
                            nc.tensor.matmul(
                                k_ps,
                                lhsT=wqkv_sb[:, c, D + m * P : D + (m + 1) * P],
                                rhs=xbT_sb[:, c, ts(q, 512)],
                                start=(c == 0), stop=(c == DC - 1),
                                skip_group_check=True,
                            )
                        nc.vector.tensor_scalar_add(
                            KT_sb[:, m, ts(q, 512)], k_ps, bqkvT_sb[:, 4 + m : 5 + m],
                        )

                    def emit_q(m, hf):
                        s5 = ts(hf, 512)
                        q2 = psKV.tile([P, 512], f32, name="q_ps2", tag="vk")
                        for c in range(DC):
                            nc.tensor.matmul(
                                q2,
                                lhsT=wqkv_sb[:, c, ts(m, P)],
                                rhs=xqT_sb[:, c, s5],
                                start=(c == 0), stop=(c == DC - 1),
                                skip_group_check=True,
                            )
                        nc.vector.tensor_scalar_add(
                            QT_sb[:, m, s5], q2, bqkvT_sb[:, m : m + 1],
                        )

                    # filler schedule: remaining projections paced under the
                    # exp-bound attention stream, each finishing just before
                    # its first consumer
                    fill = {}
                    for hh in range(1, DC):
                        fill[(hh, 3)] = lambda m=hh: emit_k(m, 0)
                        fill[(hh, 7)] = lambda m=hh: emit_k(m, 1)
                        fill[(hh, 11)] = lambda m=hh: emit_k(m, 2)
                        fill[(hh, 15)] = lambda m=hh: emit_k(m, 3)
                        fill[(hh, 5)] = lambda m=hh: emit_q(m, 0)
                        fill[(hh, 9)] = lambda m=hh: emit_q(m, 1)
                    fill[(0, 3)] = lambda: emit_k(0, 1)
                    fill[(0, 7)] = lambda: emit_k(0, 2)
                    fill[(0, 11)] = lambda: emit_k(0, 3)

                    def emit_scores(h, kc):
                        c4, r64 = h // 2, (h % 2) * HD
                        sc_ps = psSc.tile([P, TOK], f32, name="sc_ps", tag="sc")
                        for sl in range(NSL):
                            nc.tensor.matmul(
                                sc_ps[:, ts(sl, 512)],
                                lhsT=KT_sb[r64 : r64 + HD, c4, ts(kc, P)],
                                rhs=QT_sb[r64 : r64 + HD, c4, ts(sl, 512)],
                                start=True, stop=True, skip_group_check=True,
                            )
                        expT = expP.tile([P, TOK], bf16, name="expT",
                                         tag="expT")
                        nc.scalar.activation(expT, sc_ps, AF.Exp, scale=0.125)
                        return expT

                    for h in range(4):
                        c4, r64 = h // 2, (h % 2) * HD
                        ctx_ps = psCtx.tile([VW, TOK], f32, name="ctx_ps", tag="ctx")
                        exps = []

                        def emit_av(kc, e, h=h, ctx_ps=ctx_ps):
                            for sl in range(NSL):
                                nc.tensor.matmul(
                                    ctx_ps[:, ts(sl, 512)],
                                    lhsT=V_sb[:, kc, h * VW : (h + 1) * VW],
                                    rhs=e[:, ts(sl, 512)],
                                    start=(kc == 0), stop=(kc == SC - 1),
                                    skip_group_check=True,
                                )

                        # head 0 lags V by 2 and AV by 3 so the V weights'
                        # DMA latency never blocks the score/exp stream
                        for kc in range(SC):
                            exps.append(emit_scores(h, kc))
                            if h == 0 and kc >= 2:
                                emit_v(kc - 2)
                            f = fill.get((h, kc))
                            if f is not None:
                                f()
                            lag = 3 if h == 0 else 1
                            if kc >= lag:
                                emit_av(kc - lag, exps[kc - lag])
                        if h == 0:
                            emit_v(SC - 2)
                            emit_v(SC - 1)
                            for kc in range(SC - 3, SC):
                                emit_av(kc, exps[kc])
                        else:
                            emit_av(SC - 1, exps[SC - 1])

                        # psum reads (1 copy + 2 recips) free the ctx bank for
                        # the next head ~1.5us after the last AV; the
                        # normalize runs off the SBUF copy, off the AV path
                        ctxc = bcP.tile([P, TOK], f32, name="ctxc", tag="ctxc")
                        nc.vector.tensor_copy(
                            ctxc[r64 : r64 + HD, :], ctx_ps[0:HD, :])
                        recips = []
                        for hf in range(NSL):
                            s5 = ts(hf, 512)
                            recip = bcP.tile([1, 512], f32, name="recip",
                                             tag="recip")
                            nc.vector.reciprocal(recip, ctx_ps[HD : HD + 1, s5])
                            recips.append(recip)
                        for hf in range(NSL):
                            s5 = ts(hf, 512)
                            bc_sb = bcP.tile([P, 512], f32, name="bc_sb", tag="bc")
                            nc.gpsimd.partition_broadcast(bc_sb, recips[hf])
                            nc.vector.tensor_tensor(
                                ctxT_sb[r64 : r64 + HD, c4, s5],
                                ctxc[r64 : r64 + HD, s5], bc_sb[r64 : r64 + HD, :],
                                op=ALU.mult,
                            )

                    kv_scope.close()  # frees the filler + ctx psum banks

                    # ---- heads 4-7: natural-layout AV with exp stationary
                    # (full 128-deep contraction: 520 cycles/kc vs 1024) and
                    # a per-partition softmax denominator (no Pool broadcast);
                    # ctx comes back via a PE transpose per query chunk ----
                    with tc.tile_pool(name="avn_ps", bufs=1,
                                      space="PSUM") as psAVN, \
                         tc.tile_pool(name="tr_ps", bufs=1,
                                      space="PSUM") as psTR:

                        def emit_norm(h, avn_ps, qc):
                            c4, r64 = h // 2, (h % 2) * HD
                            recip = bcP.tile([P, 1], f32, name="recn",
                                             tag="recn")
                            nc.vector.reciprocal(
                                recip, avn_ps[:, qc, HD : HD + 1])
                            ctxn = bcP.tile([P, HD], bf16, name="ctxn",
                                            tag="ctxn")
                            nc.vector.tensor_scalar_mul(
                                ctxn, avn_ps[:, qc, 0:HD], recip)
                            tr = psTR.tile([HD, P], bf16, name="tr", tag="tr")
                            nc.tensor.transpose(tr, ctxn, ident_sb[:])
                            nc.vector.tensor_copy(
                                ctxT_sb[r64 : r64 + HD, c4, ts(qc, P)], tr)

                        prev = None
                        for h in range(4, H):
                            avn_ps = psAVN.tile([P, 8, VW], f32, name="avn",
                                                tag="avn")
                            exps = []

                            def emit_av2(kc, e, h=h, avn_ps=avn_ps):
                                for qc in range(8):
                                    nc.tensor.matmul(
                                        avn_ps[:, qc, :],
                                        lhsT=e[:, ts(qc, P)],
                                        rhs=V_sb[:, kc, h * VW : (h + 1) * VW],
                                        start=(kc == 0), stop=(kc == SC - 1),
                                        skip_group_check=True,
                                    )

                            for kc in range(SC):
                                exps.append(emit_scores(h, kc))
                                # previous head's normalize interleaves here
                                # so its psum release never stalls the PE;
                                # lag-3 AVs give the DVE chains time to free
                                # the single avn buffer
                                if prev is not None and kc < 8:
                                    emit_norm(prev[0], prev[1], kc)
                                if kc >= 3:
                                    emit_av2(kc - 3, exps[kc - 3])
                            for kc in range(SC - 3, SC):
                                emit_av2(kc, exps[kc])
                            prev = (h, avn_ps)
                        for qc in range(8):
                            emit_norm(H - 1, prev[1], qc)

            # ---- Wo + residual + LN1, processed per token half: half 0's
            # stats/chain/combine overlap half 1's Wo matmuls on the PE ----
            with tc.tile_pool(name="wo_ps", bufs=3, space="PSUM") as psWo, \
                 tc.tile_pool(name="ln1s_ps", bufs=1, space="PSUM") as psS1:
                sum1_ps = psS1.tile([1, TOK], f32, name="ln1_sum")
                sq1_ps = psS1.tile([1, TOK], f32, name="ln1_sqsum")
                for hf in range(NSL):
                    s5 = ts(hf, 512)

                    def _wo_finish(m, wo_ps, s5=s5):
                        nc.tensor.matmul(
                            wo_ps,
                            lhsT=wo_sb[:, DC - 1, ts(m, P)],
                            rhs=ctxT_sb[:, DC - 1, s5],
                            start=False, stop=True, skip_group_check=True,
                        )
                        nc.vector.scalar_tensor_tensor(
                            out=res1_sb[:, m, s5], in0=wo_ps,
                            scalar=boT_sb[:, m : m + 1], in1=xqT_sb[:, m, s5],
                            op0=ALU.add, op1=ALU.add,
                        )
                        sq_t = attnS.tile([P, 512], bf16, name="sq1",
                                          tag="sq1", bufs=2)
                        _ln_stats_seg(nc, sum1_ps, sq1_ps, res1_sb, sq_t,
                                      ones128, m, s5)

                    # c<3 chunks only need heads 0-5: accumulate three open
                    # groups while head 7's normalize chains drain, then
                    # append the c=3 chunk to each
                    tiles = []
                    for m in range(DC - 1):
                        wo_ps = psWo.tile([P, 512], f32, name="wo_ps", tag="wo")
                        tiles.append(wo_ps)
                        for c in range(DC - 1):
                            nc.tensor.matmul(
                                wo_ps,
                                lhsT=wo_sb[:, c, ts(m, P)],
                                rhs=ctxT_sb[:, c, s5],
                                start=(c == 0), stop=False,
                                skip_group_check=True,
                            )
                    for m in range(DC - 1):
                        _wo_finish(m, tiles[m])
                    wo_ps = psWo.tile([P, 512], f32, name="wo_ps", tag="wo")
                    for c in range(DC - 1):
                        nc.tensor.matmul(
                            wo_ps,
                            lhsT=wo_sb[:, c, ts(DC - 1, P)],
                            rhs=ctxT_sb[:, c, s5],
                            start=(c == 0), stop=False, skip_group_check=True,
                        )
                    _wo_finish(DC - 1, wo_ps)
                    segs = [(hf * 512, 256), (hf * 512 + 256, 256)]
                    _ln_chain(nc, ln1t, sum1_ps, sq1_ps, eps_sb, segs)
                    _ln_combine(nc, ln1t, attnS, res1_sb, ln1T_sb, None,
                                "ln1", [(hf * 512, 512)])

        # FFN1 consumes w = (res1-mu)*rstd directly (g1 folded into W1
        # host-side); the g1-scaled copy needed by the FFN2 residual is
        # produced by deferred tensor_scalar ops during the PE-bound FFN1.
        ffnE = top.enter_context(tc.tile_pool(name="ffnE", bufs=1))
        hid_sb = ffnE.tile([P, FC, TOK], bf16)
        ln1g_sb = ffnE.tile([P, DC, TOK], bf16)

        with tc.tile_pool(name="f1_ps", bufs=3, space="PSUM") as psF1:
            for m in range(FC):
                h_ps = psF1.tile([P, TOK], f32, name="h_ps", tag="h")
                for sl in range(4):
                    for c in range(DC):
                        nc.tensor.matmul(
                            h_ps[:, ts(sl, 256)],
                            lhsT=w1_sb[:, c, ts(m, P)],
                            rhs=ln1T_sb[:, c, ts(sl, 256)],
                            start=(c == 0), stop=(c == DC - 1),
                            skip_group_check=True,
                        )
                nc.scalar.activation(
                    hid_sb[:, m, :], h_ps, AF.Relu, bias=b1pT_sb[:, m : m + 1]
                )
            # deferred residual copy: ln1g = w * g1 (queues behind the
            # combines on DVE, runs while FFN1 owns the PE)
            for c in range(DC):
                nc.vector.tensor_scalar_mul(
                    ln1g_sb[:, c, :], ln1T_sb[:, c, :], g1T_sb[:, c : c + 1])

        # ---------------- phase 3: FFN2 + LN2 ----------------
        with tc.tile_pool(name="ffn_stat", bufs=1) as statF, \
             tc.tile_pool(name="ffn_work", bufs=2) as workF:
            res2_sb = ffnE.tile([P, DC, TOK], bf16)
            out_sb = ffnE.tile([P, DC, TOK], bf16)
            ln2t = _ln_alloc(nc, statF, "ln2")

            def ln2_out(c, s5):
                # alternate the g/beta apply between DVE and Pool so the
                # tail's combine stream isn't DVE-serial
                eng = nc.vector if c % 2 == 0 else nc.gpsimd
                eng.tensor_scalar(
                    out_sb[:, c, s5], ln1T_sb[:, c, s5], g2T_sb[:, c : c + 1],
                    beta2T_sb[:, c : c + 1], ALU.mult, ALU.add)
                nc.sync.dma_start(
                    out=outT_d[:, c, s5], in_=out_sb[:, c, s5])

            # FFN2 per token half: half 0's LN2 chain/combine/output DMA all
            # overlap half 1's 13.7us of FFN2 matmuls on the PE. ln1T is dead
            # after the deferred ln1g copies; reused as the LN2 w-scratch.
            with tc.tile_pool(name="f2_ps", bufs=3, space="PSUM") as psF2, \
                 tc.tile_pool(name="ln2s_ps", bufs=1, space="PSUM") as psS2:
                sum2_ps = psS2.tile([1, TOK], f32, name="ln2_sum")
                sq2_ps = psS2.tile([1, TOK], f32, name="ln2_sqsum")
                for hf in range(NSL):
                    s5 = ts(hf, 512)
                    for m in range(DC):
                        f_ps = psF2.tile([P, 512], f32, name="f_ps", tag="f")
                        for c in range(FC):
                            nc.tensor.matmul(
                                f_ps,
                                lhsT=w2_sb[:, c, ts(m, P)],
                                rhs=hid_sb[:, c, s5],
                                start=(c == 0), stop=(c == FC - 1),
                                skip_group_check=True,
                            )
                        nc.vector.scalar_tensor_tensor(
                            out=res2_sb[:, m, s5], in0=f_ps,
                            scalar=b2pT_sb[:, m : m + 1], in1=ln1g_sb[:, m, s5],
                            op0=ALU.add, op1=ALU.add,
                        )
                        sq_t = workF.tile([P, 512], bf16, name="sq2",
                                          tag="sq2", bufs=2)
                        _ln_stats_seg(nc, sum2_ps, sq2_ps, res2_sb, sq_t,
                                      ones128, m, s5)
                    segs = [(hf * 512, 256), (hf * 512 + 256, 256)]
                    _ln_chain(nc, ln2t, sum2_ps, sq2_ps, eps_sb, segs)
                    _ln_combine(nc, ln2t, workF, res2_sb, ln1T_sb, ln2_out,
                                "ln2", [(hf * 512, 512)])

    if not nc.is_finalized():
        nc.finalize()
    return nc


def _prep_inputs(x, Wqkv, bqkv, Wo, bo, g1, beta1, W1, b1, W2, b2, g2, beta2):
    """Host-side sharding/layout prep -> list of 8 in_maps."""
    f = lambda a: np.ascontiguousarray(np.asarray(a, dtype=np.float32))
    bf = lambda a: np.ascontiguousarray(
        np.asarray(a, dtype=np.float32).astype(ml_dtypes.bfloat16)
    )

    def chunkT(w, nchunk, cast):  # [n*128, cols] -> [128, n, cols]
        w = np.asarray(w, dtype=np.float32)
        return cast(w.reshape(nchunk, P, w.shape[1]).transpose(1, 0, 2))

    b1p = np.asarray(b1, np.float32) + np.asarray(beta1, np.float32) @ np.asarray(W1, np.float32)
    b2p = np.asarray(b2, np.float32) + np.asarray(beta1, np.float32)
    shared = {
        "wqkv": chunkT(Wqkv, DC, bf),
        "wo": chunkT(Wo, DC, bf),
        "w1": chunkT(np.asarray(W1, np.float32)
                     * np.asarray(g1, np.float32)[:, None], DC, bf),
        "w2": chunkT(W2, FC, bf),
        "bqkvT": f(np.asarray(bqkv).reshape(12, P).T),
        "boT": f(np.asarray(bo).reshape(DC, P).T),
        "b1pT": f(b1p.reshape(FC, P).T),
        "b2pT": f(b2p.reshape(DC, P).T),
        "bvrow": f(np.asarray(bqkv)[2 * D : 3 * D].reshape(1, D)),
        "ident": np.eye(P, dtype=ml_dtypes.bfloat16),
        "ones_col": np.ones((P, 1), ml_dtypes.bfloat16),
        "g1T": f(np.asarray(g1).reshape(DC, P).T),
        "g2T": f(np.asarray(g2).reshape(DC, P).T),
        "beta2T": f(np.asarray(beta2).reshape(DC, P).T),
    }
    x = np.asarray(x, dtype=np.float32)
    in_maps = []
    for c in range(8):
        b, half = c // 2, c % 2
        xbT = x[b].T.reshape(DC, P, S).transpose(1, 0, 2)
        xq = x[b, half * TOK : (half + 1) * TOK]
        xqT = xq.T.reshape(DC, P, TOK).transpose(1, 0, 2)
        in_maps.append(dict(shared, xbT=bf(xbT), xqT=bf(xqT)))
    return in_maps


def kernel(**inputs):
    from concourse.bass_utils import run_bass_kernel_spmd

    nc = _build_program()
    in_maps = _prep_inputs(**inputs)
    res = run_bass_kernel_spmd(nc, in_maps, core_ids=list(range(8)))
    out = np.empty((B, S, D), dtype=np.float32)
    for c in range(8):
        b, half = c // 2, c % 2
        oT = np.asarray(res.results[c]["outT"], dtype=np.float32)  # [P, DC, TOK]
        out[b, half * TOK : (half + 1) * TOK] = (
            oT.transpose(2, 1, 0).reshape(TOK, D)
        )
    return out



# revision 5
# speedup vs baseline: 1.1058x; 1.1058x over previous
"""Trainium2 Bass kernel for a dense transformer encoder layer.

Shapes: B=4, S=2048, D=512, H=8 heads (HD=64), FFN F=2048.

Sharding (8 NeuronCores, no collectives): core c handles batch b = c//2 and
query-half half = c%2 (1024 query tokens); K/V are computed for the full
2048-token sequence on both cores of a pair.

Design notes:
- The attention phase is ACT-bound: the 128 softmax exp ops ([128,1024] each)
  set a ~135us floor. Everything else is scheduled under that roof.
- All attention-path matmuls (QKV proj, scores, AV, Wo) and FFN1 run as fp8e4
  DoubleRow matmuls (2 contraction tiles per instruction, 0.5 cycles/row).
  Scores use a duplicated-plane trick (both planes hold K/sqrt2, Q/sqrt2) to
  fill the 2x64 contraction; projections pair genuine 128-deep c-chunks.
- AV keeps exp stationary-free: lhsT = V8 [key,2,65] over kc pairs, rhs = the
  exp pair [128,2,1024]; the softmax denominator rides in V column 64.
- The residual spine (res1/res2) is fp32 to buy error headroom for fp8;
  LayerNorm stats run off bf16 copies via ones-matmuls.
- FFN2 stays bf16 (fp8 there costs too much accuracy).
- Host transposes x / un-transposes the output.
"""

import functools
import numpy as np
from contextlib import ExitStack

import ml_dtypes

import concourse.bass as bass
import concourse.tile as tile
import concourse.mybir as mybir
from concourse import bacc
from concourse.bass import ts
from concourse.vector_clock import ScopedClock

B, S, D, H, F = 4, 2048, 512, 8, 2048
HD = D // H           # 64
P = 128
DC = D // P           # 4   d chunks
FC = F // P           # 16  ffn chunks
SC = S // P           # 16  key chunks
KCP = SC // 2         # 8   key-chunk pairs
TOK = S // 2          # 1024 query tokens per core
EPS = 1e-5
VW = HD + 2           # 66: V cols per head incl. ones + pad (DR needs even)

f32 = mybir.dt.float32
bf16 = mybir.dt.bfloat16
f8 = mybir.dt.float8e4
AF = mybir.ActivationFunctionType
ALU = mybir.AluOpType
DR = mybir.MatmulPerfMode.DoubleRow

F8NP = ml_dtypes.float8_e4m3


class _TC(tile.TileContext):
    """TileContext whose tail drain splits sem waits one-per-drain: the
    walrus build in this container rejects >1 sync wait on an SP TPB_CTRL."""

    def _drain_and_barrier(self, tick_clock, wait_clock):
        nc = self.nc
        drain_inst = nc.sync.drain()
        wait_clock.add_sem_waits(
            drain_inst.ins, ScopedClock({None: tick_clock.global_clock})
        )
        si = drain_inst.ins.sync_info
        waits = list(si.on_wait) if si and si.on_wait else []
        MAXW = 1
        if len(waits) > MAXW:
            si.on_wait = waits[:MAXW]
            for i in range(MAXW, len(waits), MAXW):
                extra = nc.sync.drain()
                extra.ins.sync_info = mybir.SyncInfo(
                    on_wait=waits[i : i + MAXW], on_update=[]
                )
        nc.all_engine_barrier()
        popped = nc._tile_sem_poison_stack.pop()
        assert popped is self._sem_poison
        nc.clear_and_free_semaphores(list(self.sems.allocated().values()))
        nc.all_engine_barrier()


def _bcast_ap(row_ap, nparts):
    """AP reading the single-partition row `row_ap` broadcast to nparts."""
    return bass.AP(
        tensor=row_ap.tensor,
        offset=row_ap.offset,
        ap=[[0, nparts]] + [list(d) for d in row_ap.ap[1:]],
    )


def _ln_alloc(nc, stat_pool, tagp):
    return {
        "mu": stat_pool.tile([1, TOK], f32, name=f"{tagp}_mu", tag=f"{tagp}_mu"),
        "tB": stat_pool.tile([1, TOK], f32, name=f"{tagp}_tB", tag=f"{tagp}_tB"),
        "var": stat_pool.tile([1, TOK], f32, name=f"{tagp}_var", tag=f"{tagp}_var"),
        "a": stat_pool.tile([1, TOK], f32, name=f"{tagp}_a", tag=f"{tagp}_a"),
        "b": stat_pool.tile([1, TOK], f32, name=f"{tagp}_b", tag=f"{tagp}_b"),
        "abf": stat_pool.tile([1, TOK], bf16, name=f"{tagp}_abf",
                              tag=f"{tagp}_abf"),
        "bbf": stat_pool.tile([1, TOK], bf16, name=f"{tagp}_bbf",
                              tag=f"{tagp}_bbf"),
        "bcA": stat_pool.tile([P, TOK], bf16, name=f"{tagp}_bcA",
                              tag=f"{tagp}_bcA"),
        "bcB": stat_pool.tile([P, TOK], bf16, name=f"{tagp}_bcB",
                              tag=f"{tagp}_bcB"),
    }


def _ln_chain(nc, t, sum_ps, sq_ps, eps_sb, segs):
    """Per token segment: a = 1/sqrt(var+eps), b = -mu*a, then broadcast bf16
    copies of (a, b) to all 128 partitions (Pool)."""
    for off, w in segs:
        s5 = slice(off, off + w)
        nc.scalar.activation(t["tB"][:, s5], sum_ps[:, s5], AF.Square,
                             scale=1.0 / D)
        nc.vector.tensor_scalar_mul(t["mu"][:, s5], sum_ps[:, s5], -1.0 / D)
        nc.vector.scalar_tensor_tensor(
            out=t["var"][:, s5], in0=sq_ps[:, s5], scalar=1.0 / D,
            in1=t["tB"][:, s5], op0=ALU.mult, op1=ALU.subtract,
        )
        nc.scalar.activation(t["tB"][:, s5], t["var"][:, s5], AF.Sqrt,
                             bias=eps_sb, scale=1.0)
        nc.vector.reciprocal(t["a"][:, s5], t["tB"][:, s5])
        nc.vector.tensor_mul(t["b"][:, s5], t["mu"][:, s5], t["a"][:, s5])
        nc.scalar.copy(t["abf"][:, s5], t["a"][:, s5])
        nc.scalar.copy(t["bbf"][:, s5], t["b"][:, s5])
        nc.gpsimd.partition_broadcast(t["bcA"][:, s5], t["abf"][:, s5])
        nc.gpsimd.partition_broadcast(t["bcB"][:, s5], t["bbf"][:, s5])


@functools.lru_cache(maxsize=1)
def _build_program():
    nc = bacc.Bacc()

    def dp(name, shape, out=False, dt=f32):
        return nc.declare_dram_parameter(name, list(shape), dt, isOutput=out)

    # fp8 activations/weights, c-chunk index split as c = 2*cp + i
    x8q_d = dp("x8q", [P, 2, 2, TOK], dt=f8)
    x8b_d = dp("x8b", [P, 2, 2, S], dt=f8)
    xq_d = dp("xq", [P, DC, TOK], dt=bf16)      # residual spine input
    wq8_d = dp("wq8", [P, 2, 2, D], dt=f8)      # Wq/sqrt2
    wk8_d = dp("wk8", [P, 2, 2, D], dt=f8)      # Wk/sqrt2
    wv8_d = dp("wv8", [P, 2, 2, D], dt=f8)
    wo8_d = dp("wo8", [P, 2, 2, D], dt=f8)
    w18_d = dp("w18", [P, 2, 2, F], dt=f8)      # 16*g1*W1
    w2_d = dp("w2", [P, FC, D], dt=bf16)
    bqkvT_d = dp("bqkvT", [P, 12])              # q,k quarters pre-scaled
    boT_d = dp("boT", [P, DC])
    b1pT_d = dp("b1pT", [P, FC])
    b2pT_d = dp("b2pT", [P, DC])
    bvrow_d = dp("bvrow", [1, D])
    ones_col_d = dp("ones_col", [P, 1], dt=bf16)
    g1T_d = dp("g1T", [P, DC])
    g2T_d = dp("g2T", [P, DC])
    beta2T_d = dp("beta2T", [P, DC])
    outT_d = dp("outT", [P, DC, TOK], out=True, dt=bf16)

    with _TC(nc) as tc, ExitStack() as top:
        top.enter_context(
            nc.allow_low_precision(reason="fp8/bf16 matmul pipeline by design")
        )
        persist = top.enter_context(tc.tile_pool(name="persist", bufs=1))
        bqkvT_sb = persist.tile([P, 12], f32)
        boT_sb = persist.tile([P, DC], f32)
        b1pT_sb = persist.tile([P, FC], f32)
        b2pT_sb = persist.tile([P, DC], f32)
        g1T_sb = persist.tile([P, DC], f32)
        g2T_sb = persist.tile([P, DC], f32)
        beta2T_sb = persist.tile([P, DC], f32)
        bvb_sb = persist.tile([P, D], f32)
        ones128 = persist.tile([P, 1], bf16)
        eps_sb = persist.tile([1, 1], f32)
        nc.vector.memset(eps_sb, EPS)

        # weights + x (whole kernel lifetime)
        wP = top.enter_context(tc.tile_pool(name="wP", bufs=1))
        x8q_sb = wP.tile([P, 2, 2, TOK], f8)
        x8b_sb = wP.tile([P, 2, 2, S], f8)
        xq_sb = wP.tile([P, DC, TOK], bf16)
        wq8_sb = wP.tile([P, 2, 2, D], f8)
        wk8_sb = wP.tile([P, 2, 2, D], f8)
        wv8_sb = wP.tile([P, 2, 2, D], f8)
        wo8_sb = wP.tile([P, 2, 2, D], f8)
        w18_sb = wP.tile([P, 2, 2, F], f8)
        w2_sb = wP.tile([P, FC, D], bf16)

        # survives into the post phase
        mid = top.enter_context(tc.tile_pool(name="mid", bufs=1))
        ctxT8_sb = mid.tile([P, 2, 2, TOK], f8)   # [d-part, cp, i, tok]
        spine_sb = mid.tile([P, DC, TOK], f32)    # res1, later res2

        # ---- DMA queue: need order ----
        nc.sync.dma_start(out=wq8_sb, in_=wq8_d[:])
        nc.sync.dma_start(out=x8q_sb, in_=x8q_d[:])
        nc.sync.dma_start(out=wk8_sb, in_=wk8_d[:])
        nc.sync.dma_start(out=x8b_sb[:, :, :, 0:512], in_=x8b_d[:, :, :, 0:512])
        nc.sync.dma_start(out=wv8_sb, in_=wv8_d[:])
        nc.sync.dma_start(out=bqkvT_sb, in_=bqkvT_d[:])
        nc.gpsimd.dma_start(out=bvb_sb, in_=_bcast_ap(bvrow_d[:], P))
        nc.sync.dma_start(out=ones128, in_=ones_col_d[:])
        for q in range(1, 4):
            nc.sync.dma_start(out=x8b_sb[:, :, :, ts(q, 512)],
                              in_=x8b_d[:, :, :, ts(q, 512)])
        nc.sync.dma_start(out=boT_sb, in_=boT_d[:])
        nc.sync.dma_start(out=b1pT_sb, in_=b1pT_d[:])
        nc.sync.dma_start(out=b2pT_sb, in_=b2pT_d[:])
        nc.sync.dma_start(out=g1T_sb, in_=g1T_d[:])
        nc.sync.dma_start(out=g2T_sb, in_=g2T_d[:])
        nc.sync.dma_start(out=beta2T_sb, in_=beta2T_d[:])
        nc.sync.dma_start(out=wo8_sb, in_=wo8_d[:])
        nc.sync.dma_start(out=xq_sb, in_=xq_d[:])
        nc.sync.dma_start(out=w18_sb, in_=w18_d[:])
        for c in range(0, FC, 8):
            nc.sync.dma_start(out=w2_sb[:, c : c + 8, :],
                              in_=w2_d[:, c : c + 8, :])

        with ExitStack() as attn_scope:
            attnP = attn_scope.enter_context(tc.tile_pool(name="attnP", bufs=1))
            Q8_sb = attnP.tile([P, 2, 4, TOK], f8)    # [64(h%2)+hd, pl, j, tok]
            K8_sb = attnP.tile([P, 2, 4, S], f8)
            V8_sb = attnP.tile([P, KCP, 2, H, VW], f8)
            nc.gpsimd.memset(V8_sb[:, :, :, :, HD:VW], 1.0)

            psFill = attn_scope.enter_context(
                tc.tile_pool(name="fill_ps", bufs=2, space="PSUM"))
            psSc = attn_scope.enter_context(
                tc.tile_pool(name="sc_ps", bufs=2, space="PSUM"))
            psCtx = attn_scope.enter_context(
                tc.tile_pool(name="ctx_ps", bufs=1, space="PSUM"))
            expP = attn_scope.enter_context(tc.tile_pool(name="expP", bufs=3))
            nrmP = attn_scope.enter_context(tc.tile_pool(name="nrmP", bufs=2))

            def emit_q(j, th):
                s5 = ts(th, 512)
                q_ps = psFill.tile([P, 512], f32, name="q_ps", tag="fill")
                for cp in range(2):
                    nc.tensor.matmul(
                        q_ps,
                        lhsT=wq8_sb[:, cp, :, ts(j, P)],
                        rhs=x8q_sb[:, cp, :, s5],
                        start=(cp == 0), stop=(cp == 1),
                        perf_mode=DR, skip_group_check=True,
                    )
                for pl in range(2):
                    nc.vector.tensor_scalar_add(
                        Q8_sb[:, pl, j, s5], q_ps, bqkvT_sb[:, j : j + 1])

            def emit_k(j, q):
                s5 = ts(q, 512)
                k_ps = psFill.tile([P, 512], f32, name="k_ps", tag="fill")
                for cp in range(2):
                    nc.tensor.matmul(
                        k_ps,
                        lhsT=wk8_sb[:, cp, :, ts(j, P)],
                        rhs=x8b_sb[:, cp, :, s5],
                        start=(cp == 0), stop=(cp == 1),
                        perf_mode=DR, skip_group_check=True,
                    )
                for pl in range(2):
                    nc.vector.tensor_scalar_add(
                        K8_sb[:, pl, j, s5], k_ps, bqkvT_sb[:, 4 + j : 5 + j])

            bvb_h = bvb_sb.rearrange("p (h e) -> p h e", e=HD)

            def emit_v(kc):
                v_ps = psFill.tile([P, D], f32, name="v_ps", tag="fill")
                for cp in range(2):
                    nc.tensor.matmul(
                        v_ps,
                        lhsT=x8b_sb[:, cp, :, ts(kc, P)],
                        rhs=wv8_sb[:, cp, :, :],
                        start=(cp == 0), stop=(cp == 1),
                        perf_mode=DR, skip_group_check=True,
                    )
                nc.vector.tensor_tensor(
                    V8_sb[:, kc // 2, kc % 2, :, 0:HD],
                    v_ps.rearrange("p (h e) -> p h e", e=HD),
                    bvb_h, op=ALU.add,
                )

            # fill schedule: (h, kc) -> list of closures
            fill = {}
            fill[(0, 1)] = [lambda: emit_k(0, 1)]
            fill[(0, 5)] = [lambda: emit_k(0, 2)]
            fill[(0, 9)] = [lambda: emit_k(0, 3)]
            fill[(0, 13)] = [lambda: emit_q(1, 0)]
            fill[(0, 14)] = [lambda: emit_q(1, 1)]
            fill[(1, 1)] = [lambda: emit_k(1, 0)]
            fill[(1, 3)] = [lambda: emit_k(1, 1)]
            fill[(1, 5)] = [lambda: emit_k(1, 2)]
            fill[(1, 7)] = [lambda: emit_k(1, 3)]
            fill[(1, 9)] = [lambda: emit_q(2, 0)]
            fill[(1, 11)] = [lambda: emit_q(2, 1)]
            fill[(2, 3)] = [lambda: emit_k(2, 0)]
            fill[(2, 7)] = [lambda: emit_k(2, 1)]
            fill[(2, 11)] = [lambda: emit_k(2, 2)]
            fill[(2, 15)] = [lambda: emit_k(2, 3)]
            fill[(3, 5)] = [lambda: emit_q(3, 0)]
            fill[(3, 9)] = [lambda: emit_q(3, 1)]
            fill[(4, 3)] = [lambda: emit_k(3, 0)]
            fill[(4, 7)] = [lambda: emit_k(3, 1)]
            fill[(4, 11)] = [lambda: emit_k(3, 2)]
            fill[(4, 15)] = [lambda: emit_k(3, 3)]

            # upfront projections for head 0
            emit_q(0, 0)
            emit_q(0, 1)
            emit_k(0, 0)

            for h in range(H):
                j, hb = h // 2, 64 * (h % 2)
                ctx_ps = psCtx.tile([VW, TOK], f32, name="ctx_ps", tag="ctx")
                exps = []

                def emit_av(kcp_, e, h=h, ctx_ps=ctx_ps):
                    for th in range(2):
                        nc.tensor.matmul(
                            ctx_ps[:, ts(th, 512)],
                            lhsT=V8_sb[:, kcp_, :, h, :],
                            rhs=e[:, :, ts(th, 512)],
                            start=(kcp_ == 0), stop=(kcp_ == KCP - 1),
                            perf_mode=DR, skip_group_check=True,
                        )

                exp_t = None
                for kc in range(SC):
                    for f_ in fill.get((h, kc), ()):
                        f_()
                    if h == 0 and kc < 14:
                        emit_v(kc)
                    if kc % 2 == 0:
                        exp_t = expP.tile([P, 2, TOK], f8, name="exp8",
                                          tag="exp8")
                        exps.append(exp_t)
                    sc_ps = psSc.tile([P, TOK], f32, name="sc_ps", tag="sc")
                    for th in range(2):
                        nc.tensor.matmul(
                            sc_ps[:, ts(th, 512)],
                            lhsT=K8_sb[hb : hb + HD, :, j, ts(kc, P)],
                            rhs=Q8_sb[hb : hb + HD, :, j, ts(th, 512)],
                            start=True, stop=True,
                            perf_mode=DR, skip_group_check=True,
                        )
                    nc.scalar.activation(exp_t[:, kc % 2, :], sc_ps, AF.Exp,
                                         scale=0.125)
                    if kc % 2 == 1 and kc >= 3:
                        emit_av(kc // 2 - 1, exps[kc // 2 - 1])
                if h == 0:
                    emit_v(14)
                    emit_v(15)
                emit_av(KCP - 1, exps[KCP - 1])

                # normalize off-psum: copy out, recip row 64, bcast, scale
                ctmp = nrmP.tile([VW, TOK], f32, name="ctmp", tag="ctmp")
                nc.vector.tensor_copy(ctmp, ctx_ps)
                rden = nrmP.tile([1, TOK], f32, name="rden", tag="rden")
                nc.vector.reciprocal(rden, ctmp[HD : HD + 1, :])
                rb = nrmP.tile([HD, TOK], f32, name="rb", tag="rb")
                nc.gpsimd.partition_broadcast(rb, rden)
                nc.vector.tensor_tensor(
                    ctxT8_sb[hb : hb + HD, h // 4, (h // 2) % 2, :],
                    ctmp[0:HD, :], rb, op=ALU.mult,
                )

        # ---- post phase: Wo + LN1 + FFN1 (fp8 DR) + FFN2 (bf16) + LN2 ----
        postP = top.enter_context(tc.tile_pool(name="postP", bufs=1))
        ln18_sb = postP.tile([P, 2, 2, TOK], f8)
        ln1g_sb = postP.tile([P, DC, TOK], bf16)
        hid_sb = postP.tile([P, FC, TOK], bf16)
        out_sb = postP.tile([P, DC, TOK], bf16)
        workP = top.enter_context(tc.tile_pool(name="workP", bufs=2))
        ln1t = _ln_alloc(nc, postP, "ln1")
        ln2t = _ln_alloc(nc, postP, "ln2")

        def emit_stats(src_slice, sum_ps, sq_ps, s5, first, last, tagp):
            sbf = workP.tile([P, 512], bf16, name=f"{tagp}_sbf",
                             tag=f"{tagp}_sbf")
            nc.vector.tensor_copy(sbf, src_slice)
            sq = workP.tile([P, 512], bf16, name=f"{tagp}_sq", tag=f"{tagp}_sq")
            nc.vector.tensor_mul(sq, sbf, sbf)
            nc.tensor.matmul(sum_ps[:, s5], lhsT=ones128, rhs=sbf,
                             start=first, stop=last, skip_group_check=True)
            nc.tensor.matmul(sq_ps[:, s5], lhsT=ones128, rhs=sq,
                             start=first, stop=last, skip_group_check=True)

        with tc.tile_pool(name="wo_ps", bufs=3, space="PSUM") as psWo, \
             tc.tile_pool(name="ln1s_ps", bufs=1, space="PSUM") as psS1:
            sum1_ps = psS1.tile([1, TOK], f32, name="ln1_sum")
            sq1_ps = psS1.tile([1, TOK], f32, name="ln1_sqsum")
            for hf in range(2):
                s5 = ts(hf, 512)
                for m in range(DC):
                    wo_ps = psWo.tile([P, 512], f32, name="wo_ps", tag="wo")
                    for cp in range(2):
                        nc.tensor.matmul(
                            wo_ps,
                            lhsT=wo8_sb[:, cp, :, ts(m, P)],
                            rhs=ctxT8_sb[:, cp, :, s5],
                            start=(cp == 0), stop=(cp == 1),
                            perf_mode=DR, skip_group_check=True,
                        )
                    nc.vector.scalar_tensor_tensor(
                        out=spine_sb[:, m, s5], in0=wo_ps,
                        scalar=boT_sb[:, m : m + 1], in1=xq_sb[:, m, s5],
                        op0=ALU.add, op1=ALU.add,
                    )
                    emit_stats(spine_sb[:, m, s5], sum1_ps, sq1_ps, s5,
                               m == 0, m == DC - 1, "s1")
                segs = [(hf * 512, 256), (hf * 512 + 256, 256)]
                _ln_chain(nc, ln1t, sum1_ps, sq1_ps, eps_sb, segs)
                # combine: ln18 (f8, FFN1 input) + ln1g (bf16, FFN2 residual)
                for c in range(DC):
                    v = workP.tile([P, 512], bf16, name="ln1v", tag="ln1v")
                    nc.vector.tensor_mul(v, spine_sb[:, c, s5],
                                         ln1t["bcA"][:, s5])
                    t = workP.tile([P, 512], bf16, name="ln1t", tag="ln1t")
                    nc.vector.tensor_tensor(t, v, ln1t["bcB"][:, s5],
                                            op=ALU.add)
                    nc.gpsimd.tensor_copy(ln18_sb[:, c // 2, c % 2, s5], t)
                    nc.vector.tensor_scalar_mul(
                        ln1g_sb[:, c, s5], t, g1T_sb[:, c : c + 1])

        # FFN1 (fp8 DR) + relu
        with tc.tile_pool(name="f1_ps", bufs=3, space="PSUM") as psF1:
            for hf in range(2):
                s5 = ts(hf, 512)
                for m in range(FC):
                    h_ps = psF1.tile([P, 512], f32, name="h_ps", tag="h")
                    for cp in range(2):
                        nc.tensor.matmul(
                            h_ps,
                            lhsT=w18_sb[:, cp, :, ts(m, P)],
                            rhs=ln18_sb[:, cp, :, s5],
                            start=(cp == 0), stop=(cp == 1),
                            perf_mode=DR, skip_group_check=True,
                        )
                    nc.scalar.activation(
                        hid_sb[:, m, s5], h_ps, AF.Relu,
                        bias=b1pT_sb[:, m : m + 1], scale=0.0625,
                    )

        # FFN2 (bf16) + LN2, pipelined per token half
        with tc.tile_pool(name="f2_ps", bufs=3, space="PSUM") as psF2, \
             tc.tile_pool(name="ln2s_ps", bufs=1, space="PSUM") as psS2:
            sum2_ps = psS2.tile([1, TOK], f32, name="ln2_sum")
            sq2_ps = psS2.tile([1, TOK], f32, name="ln2_sqsum")
            for hf in range(2):
                s5 = ts(hf, 512)
                for m in range(DC):
                    f_ps = psF2.tile([P, 512], f32, name="f_ps", tag="f")
                    for c in range(FC):
                        nc.tensor.matmul(
                            f_ps,
                            lhsT=w2_sb[:, c, ts(m, P)],
                            rhs=hid_sb[:, c, s5],
                            start=(c == 0), stop=(c == FC - 1),
                            skip_group_check=True,
                        )
                    nc.vector.scalar_tensor_tensor(
                        out=spine_sb[:, m, s5], in0=f_ps,
                        scalar=b2pT_sb[:, m : m + 1], in1=ln1g_sb[:, m, s5],
                        op0=ALU.add, op1=ALU.add,
                    )
                    emit_stats(spine_sb[:, m, s5], sum2_ps, sq2_ps, s5,
                               m == 0, m == DC - 1, "s2")
                segs = [(hf * 512, 256), (hf * 512 + 256, 256)]
                _ln_chain(nc, ln2t, sum2_ps, sq2_ps, eps_sb, segs)
                for c in range(DC):
                    v = workP.tile([P, 512], bf16, name="ln2v", tag="ln2v")
                    nc.vector.tensor_mul(v, spine_sb[:, c, s5],
                                         ln2t["bcA"][:, s5])
                    t = workP.tile([P, 512], bf16, name="ln2t", tag="ln2t")
                    nc.vector.tensor_tensor(t, v, ln2t["bcB"][:, s5],
                                            op=ALU.add)
                    eng = nc.vector if c % 2 == 0 else nc.gpsimd
                    eng.tensor_scalar(
                        out_sb[:, c, s5], t, g2T_sb[:, c : c + 1],
                        beta2T_sb[:, c : c + 1], ALU.mult, ALU.add)
                    nc.sync.dma_start(out=outT_d[:, c, s5],
                                      in_=out_sb[:, c, s5])

    if not nc.is_finalized():
        nc.finalize()
    return nc


def _prep_inputs(x, Wqkv, bqkv, Wo, bo, g1, beta1, W1, b1, W2, b2, g2, beta2):
    """Host-side sharding/layout prep -> list of 8 in_maps."""
    f = lambda a: np.ascontiguousarray(np.asarray(a, dtype=np.float32))
    bf = lambda a: np.ascontiguousarray(
        np.asarray(a, dtype=np.float32).astype(ml_dtypes.bfloat16))
    q8 = lambda a: np.ascontiguousarray(
        np.asarray(a, dtype=np.float32).astype(F8NP))

    def pack8(w):  # [512, N] -> [128, 2, 2, N] fp8, c = 2*cp + i
        w = np.asarray(w, dtype=np.float32)
        return q8(w.reshape(2, 2, P, w.shape[1]).transpose(2, 0, 1, 3))

    def chunkT(w, nchunk, cast):  # [n*128, cols] -> [128, n, cols]
        w = np.asarray(w, dtype=np.float32)
        return cast(w.reshape(nchunk, P, w.shape[1]).transpose(1, 0, 2))

    Wqkv = np.asarray(Wqkv, np.float32)
    s2 = 1.0 / np.sqrt(2.0)
    bqkv_s = np.asarray(bqkv, np.float32).copy()
    bqkv_s[: 2 * D] *= s2                      # q,k bias pre-scaled
    b1p = np.asarray(b1, np.float32) + np.asarray(beta1, np.float32) @ np.asarray(W1, np.float32)
    b2p = np.asarray(b2, np.float32) + np.asarray(beta1, np.float32)
    shared = {
        "wq8": pack8(Wqkv[:, 0:D] * s2),
        "wk8": pack8(Wqkv[:, D : 2 * D] * s2),
        "wv8": pack8(Wqkv[:, 2 * D :]),
        "wo8": pack8(Wo),
        "w18": pack8(np.asarray(W1, np.float32)
                     * np.asarray(g1, np.float32)[:, None] * 16.0),
        "w2": chunkT(W2, FC, bf),
        "bqkvT": f(bqkv_s.reshape(12, P).T),
        "boT": f(np.asarray(bo).reshape(DC, P).T),
        "b1pT": f(b1p.reshape(FC, P).T),
        "b2pT": f(b2p.reshape(DC, P).T),
        "bvrow": f(np.asarray(bqkv, np.float32)[2 * D :].reshape(1, D)),
        "ones_col": np.ones((P, 1), ml_dtypes.bfloat16),
        "g1T": f(np.asarray(g1).reshape(DC, P).T),
        "g2T": f(np.asarray(g2).reshape(DC, P).T),
        "beta2T": f(np.asarray(beta2).reshape(DC, P).T),
    }
    x = np.asarray(x, dtype=np.float32)
    in_maps = []
    for c in range(8):
        b, half = c // 2, c % 2
        xbT = x[b].T.reshape(2, 2, P, S).transpose(2, 0, 1, 3)   # [128,2,2,S]
        xq = x[b, half * TOK : (half + 1) * TOK]
        xqT4 = xq.T.reshape(DC, P, TOK).transpose(1, 0, 2)        # [128,4,TOK]
        x8qT = xq.T.reshape(2, 2, P, TOK).transpose(2, 0, 1, 3)
        in_maps.append(dict(
            shared, x8b=q8(xbT), x8q=q8(x8qT), xq=bf(xqT4)))
    return in_maps


def kernel(**inputs):
    from concourse.bass_utils import run_bass_kernel_spmd

    nc = _build_program()
    in_maps = _prep_inputs(**inputs)
    res = run_bass_kernel_spmd(nc, in_maps, core_ids=list(range(8)))
    out = np.empty((B, S, D), dtype=np.float32)
    for c in range(8):
        b, half = c // 2, c % 2
        oT = np.asarray(res.results[c]["outT"], dtype=np.float32)  # [P,DC,TOK]
        out[b, half * TOK : (half + 1) * TOK] = (
            oT.transpose(2, 1, 0).reshape(TOK, D)
        )
    return out


# revision 8
# speedup vs baseline: 1.1356x; 1.0269x over previous
"""Trainium2 Bass kernel for a dense transformer encoder layer.

Shapes: B=4, S=2048, D=512, H=8 heads (HD=64), FFN F=2048.

Sharding (8 NeuronCores, no collectives): core c handles batch b = c//2 and
query-half half = c%2 (1024 query tokens); K/V are computed for the full
2048-token sequence on both cores of a pair.

Design notes:
- The attention phase is ACT-bound: the 128 softmax exp ops ([128,1024] each)
  set a ~135us floor. Everything else is scheduled under that roof.
- All attention-path matmuls (QKV proj, scores, AV, Wo) and FFN1 run as fp8e4
  DoubleRow matmuls (2 contraction tiles per instruction, 0.5 cycles/row).
  Scores use a duplicated-plane trick (both planes hold K/sqrt2, Q/sqrt2) to
  fill the 2x64 contraction; projections pair genuine 128-deep c-chunks.
- AV keeps exp stationary-free: lhsT = V8 [key,2,65] over kc pairs, rhs = the
  exp pair [128,2,1024]; the softmax denominator rides in V column 64.
- The residual spine (res1/res2) is fp32 to buy error headroom for fp8;
  LayerNorm stats run off bf16 copies via ones-matmuls.
- FFN2 stays bf16 (fp8 there costs too much accuracy).
- Host transposes x / un-transposes the output.
"""

import functools
import numpy as np
from contextlib import ExitStack

import ml_dtypes

import concourse.bass as bass
import concourse.tile as tile
import concourse.mybir as mybir
from concourse import bacc
from concourse.bass import ts
from concourse.vector_clock import ScopedClock

B, S, D, H, F = 4, 2048, 512, 8, 2048
HD = D // H           # 64
P = 128
DC = D // P           # 4   d chunks
FC = F // P           # 16  ffn chunks
SC = S // P           # 16  key chunks
KCP = SC // 2         # 8   key-chunk pairs
TOK = S // 2          # 1024 query tokens per core
EPS = 1e-5
VW = HD + 2           # 66: V cols per head incl. ones + pad (DR needs even)

f32 = mybir.dt.float32
bf16 = mybir.dt.bfloat16
f8 = mybir.dt.float8e4
AF = mybir.ActivationFunctionType
ALU = mybir.AluOpType
DR = mybir.MatmulPerfMode.DoubleRow

F8NP = ml_dtypes.float8_e4m3


class _TC(tile.TileContext):
    """TileContext whose tail drain splits sem waits one-per-drain: the
    walrus build in this container rejects >1 sync wait on an SP TPB_CTRL."""

    def _drain_and_barrier(self, tick_clock, wait_clock):
        nc = self.nc
        drain_inst = nc.sync.drain()
        wait_clock.add_sem_waits(
            drain_inst.ins, ScopedClock({None: tick_clock.global_clock})
        )
        si = drain_inst.ins.sync_info
        waits = list(si.on_wait) if si and si.on_wait else []
        MAXW = 1
        if len(waits) > MAXW:
            si.on_wait = waits[:MAXW]
            for i in range(MAXW, len(waits), MAXW):
                extra = nc.sync.drain()
                extra.ins.sync_info = mybir.SyncInfo(
                    on_wait=waits[i : i + MAXW], on_update=[]
                )
        nc.all_engine_barrier()
        popped = nc._tile_sem_poison_stack.pop()
        assert popped is self._sem_poison
        nc.clear_and_free_semaphores(list(self.sems.allocated().values()))
        nc.all_engine_barrier()


def _bcast_ap(row_ap, nparts):
    """AP reading the single-partition row `row_ap` broadcast to nparts."""
    return bass.AP(
        tensor=row_ap.tensor,
        offset=row_ap.offset,
        ap=[[0, nparts]] + [list(d) for d in row_ap.ap[1:]],
    )


def _ln_alloc(nc, stat_pool, tagp):
    return {
        "mu": stat_pool.tile([1, TOK], f32, name=f"{tagp}_mu", tag=f"{tagp}_mu"),
        "tB": stat_pool.tile([1, TOK], f32, name=f"{tagp}_tB", tag=f"{tagp}_tB"),
        "var": stat_pool.tile([1, TOK], f32, name=f"{tagp}_var", tag=f"{tagp}_var"),
        "a": stat_pool.tile([1, TOK], f32, name=f"{tagp}_a", tag=f"{tagp}_a"),
        "b": stat_pool.tile([1, TOK], f32, name=f"{tagp}_b", tag=f"{tagp}_b"),
        "abf": stat_pool.tile([1, TOK], bf16, name=f"{tagp}_abf",
                              tag=f"{tagp}_abf"),
        "bbf": stat_pool.tile([1, TOK], bf16, name=f"{tagp}_bbf",
                              tag=f"{tagp}_bbf"),
        "bcA": stat_pool.tile([P, TOK], bf16, name=f"{tagp}_bcA",
                              tag=f"{tagp}_bcA"),
        "bcB": stat_pool.tile([P, TOK], bf16, name=f"{tagp}_bcB",
                              tag=f"{tagp}_bcB"),
    }


def _ln_chain(nc, t, sum_ps, sq_ps, eps_sb, segs):
    """Per token segment: a = 1/sqrt(var+eps), b = -mu*a, then broadcast bf16
    copies of (a, b) to all 128 partitions (Pool)."""
    for off, w in segs:
        s5 = slice(off, off + w)
        nc.scalar.activation(t["tB"][:, s5], sum_ps[:, s5], AF.Square,
                             scale=1.0 / D)
        nc.vector.tensor_scalar_mul(t["mu"][:, s5], sum_ps[:, s5], -1.0 / D)
        nc.vector.scalar_tensor_tensor(
            out=t["var"][:, s5], in0=sq_ps[:, s5], scalar=1.0 / D,
            in1=t["tB"][:, s5], op0=ALU.mult, op1=ALU.subtract,
        )
        nc.scalar.activation(t["tB"][:, s5], t["var"][:, s5], AF.Sqrt,
                             bias=eps_sb, scale=1.0)
        nc.vector.reciprocal(t["a"][:, s5], t["tB"][:, s5])
        nc.vector.tensor_mul(t["b"][:, s5], t["mu"][:, s5], t["a"][:, s5])
        nc.scalar.copy(t["abf"][:, s5], t["a"][:, s5])
        nc.scalar.copy(t["bbf"][:, s5], t["b"][:, s5])
        nc.gpsimd.partition_broadcast(t["bcA"][:, s5], t["abf"][:, s5])
        nc.gpsimd.partition_broadcast(t["bcB"][:, s5], t["bbf"][:, s5])


@functools.lru_cache(maxsize=1)
def _build_program():
    nc = bacc.Bacc()

    def dp(name, shape, out=False, dt=f32):
        return nc.declare_dram_parameter(name, list(shape), dt, isOutput=out)

    # fp8 activations/weights, c-chunk index split as c = 2*cp + i
    x8q_d = dp("x8q", [P, 2, 2, TOK], dt=f8)
    x8b_d = dp("x8b", [P, 2, 2, S], dt=f8)
    xq_d = dp("xq", [P, DC, TOK], dt=bf16)      # residual spine input
    wq8_d = dp("wq8", [P, 2, 2, D], dt=f8)      # Wq/sqrt2
    wk8_d = dp("wk8", [P, 2, 2, D], dt=f8)      # Wk/sqrt2
    wv8_d = dp("wv8", [P, 2, 2, D], dt=f8)
    wo8_d = dp("wo8", [P, 2, 2, D], dt=f8)
    w18_d = dp("w18", [P, 2, 2, F], dt=f8)      # 16*g1*W1
    w2_d = dp("w2", [P, FC, D], dt=bf16)
    bqkvT_d = dp("bqkvT", [P, 12])              # q,k quarters pre-scaled
    boT_d = dp("boT", [P, DC])
    b1pT_d = dp("b1pT", [P, FC])
    b2pT_d = dp("b2pT", [P, DC])
    bvrow_d = dp("bvrow", [1, D])
    ones_col_d = dp("ones_col", [P, 1], dt=bf16)
    g1T_d = dp("g1T", [P, DC])
    g2T_d = dp("g2T", [P, DC])
    beta2T_d = dp("beta2T", [P, DC])
    outT_d = dp("outT", [P, DC, TOK], out=True, dt=bf16)

    with _TC(nc) as tc, ExitStack() as top:
        top.enter_context(
            nc.allow_low_precision(reason="fp8/bf16 matmul pipeline by design")
        )
        persist = top.enter_context(tc.tile_pool(name="persist", bufs=1))
        bqkvT_sb = persist.tile([P, 12], f32)
        boT_sb = persist.tile([P, DC], f32)
        b1pT_sb = persist.tile([P, FC], f32)
        b2pT_sb = persist.tile([P, DC], f32)
        g1T_sb = persist.tile([P, DC], f32)
        g2T_sb = persist.tile([P, DC], f32)
        beta2T_sb = persist.tile([P, DC], f32)
        bvb_sb = persist.tile([P, D], f32)
        ones128 = persist.tile([P, 1], bf16)
        eps_sb = persist.tile([1, 1], f32)
        nc.vector.memset(eps_sb, EPS)

        # weights + x (whole kernel lifetime)
        wP = top.enter_context(tc.tile_pool(name="wP", bufs=1))
        x8q_sb = wP.tile([P, 2, 2, TOK], f8)
        x8b_sb = wP.tile([P, 2, 2, S], f8)
        xq_sb = wP.tile([P, DC, TOK], bf16)
        wq8_sb = wP.tile([P, 2, 2, D], f8)
        wk8_sb = wP.tile([P, 2, 2, D], f8)
        wv8_sb = wP.tile([P, 2, 2, D], f8)
        wo8_sb = wP.tile([P, 2, 2, D], f8)
        w18_sb = wP.tile([P, 2, 2, F], f8)
        w2_sb = wP.tile([P, FC, D], bf16)

        # survives into the post phase
        mid = top.enter_context(tc.tile_pool(name="mid", bufs=1))
        ctxT8_sb = mid.tile([P, 2, 2, TOK], f8)   # [d-part, cp, i, tok]
        spine_sb = mid.tile([P, DC, TOK], f32)    # res1, later res2

        # ---- DMA queue: need order (biases first: they gate Q8/K8 writes) ----
        nc.sync.dma_start(out=bqkvT_sb, in_=bqkvT_d[:])
        nc.sync.dma_start(out=wq8_sb, in_=wq8_d[:])
        nc.sync.dma_start(out=x8q_sb, in_=x8q_d[:])
        nc.sync.dma_start(out=wk8_sb, in_=wk8_d[:])
        nc.sync.dma_start(out=x8b_sb[:, :, :, 0:512], in_=x8b_d[:, :, :, 0:512])
        nc.sync.dma_start(out=wv8_sb, in_=wv8_d[:])
        nc.gpsimd.dma_start(out=bvb_sb, in_=_bcast_ap(bvrow_d[:], P))
        nc.sync.dma_start(out=ones128, in_=ones_col_d[:])
        for q in range(1, 4):
            nc.sync.dma_start(out=x8b_sb[:, :, :, ts(q, 512)],
                              in_=x8b_d[:, :, :, ts(q, 512)])
        nc.sync.dma_start(out=boT_sb, in_=boT_d[:])
        nc.sync.dma_start(out=b1pT_sb, in_=b1pT_d[:])
        nc.sync.dma_start(out=b2pT_sb, in_=b2pT_d[:])
        nc.sync.dma_start(out=g1T_sb, in_=g1T_d[:])
        nc.sync.dma_start(out=g2T_sb, in_=g2T_d[:])
        nc.sync.dma_start(out=beta2T_sb, in_=beta2T_d[:])
        nc.sync.dma_start(out=wo8_sb, in_=wo8_d[:])
        nc.sync.dma_start(out=xq_sb, in_=xq_d[:])
        nc.sync.dma_start(out=w18_sb, in_=w18_d[:])
        for c in range(0, FC, 8):
            nc.sync.dma_start(out=w2_sb[:, c : c + 8, :],
                              in_=w2_d[:, c : c + 8, :])

        with ExitStack() as attn_scope:
            attnP = attn_scope.enter_context(tc.tile_pool(name="attnP", bufs=1))
            Q8_sb = attnP.tile([P, 2, 4, TOK], f8)    # [64(h%2)+hd, pl, j, tok]
            K8_sb = attnP.tile([P, 2, 4, S], f8)
            V8_sb = attnP.tile([P, KCP, 2, H, VW], f8)
            nc.gpsimd.memset(V8_sb[:, :, :, :, HD:VW], 1.0)

            psFill = attn_scope.enter_context(
                tc.tile_pool(name="fill_ps", bufs=2, space="PSUM"))
            psSc = attn_scope.enter_context(
                tc.tile_pool(name="sc_ps", bufs=2, space="PSUM"))
            psCtx = attn_scope.enter_context(
                tc.tile_pool(name="ctx_ps", bufs=1, space="PSUM"))
            expP = attn_scope.enter_context(tc.tile_pool(name="expP", bufs=3))
            nrmP = attn_scope.enter_context(tc.tile_pool(name="nrmP", bufs=2))

            def emit_q(j, th):
                s5 = ts(th, 512)
                q_ps = psFill.tile([P, 512], f32, name="q_ps", tag="fill")
                for cp in range(2):
                    nc.tensor.matmul(
                        q_ps,
                        lhsT=wq8_sb[:, cp, :, ts(j, P)],
                        rhs=x8q_sb[:, cp, :, s5],
                        start=(cp == 0), stop=(cp == 1),
                        perf_mode=DR, skip_group_check=True,
                    )
                nc.vector.tensor_scalar_add(
                    Q8_sb[:, 0, j, s5], q_ps, bqkvT_sb[:, j : j + 1])
                nc.gpsimd.tensor_copy(Q8_sb[:, 1, j, s5], Q8_sb[:, 0, j, s5])

            def emit_k(j, q):
                s5 = ts(q, 512)
                k_ps = psFill.tile([P, 512], f32, name="k_ps", tag="fill")
                for cp in range(2):
                    nc.tensor.matmul(
                        k_ps,
                        lhsT=wk8_sb[:, cp, :, ts(j, P)],
                        rhs=x8b_sb[:, cp, :, s5],
                        start=(cp == 0), stop=(cp == 1),
                        perf_mode=DR, skip_group_check=True,
                    )
                nc.vector.tensor_scalar_add(
                    K8_sb[:, 0, j, s5], k_ps, bqkvT_sb[:, 4 + j : 5 + j])
                nc.gpsimd.tensor_copy(K8_sb[:, 1, j, s5], K8_sb[:, 0, j, s5])

            bvb_h = bvb_sb.rearrange("p (h e) -> p h e", e=HD)

            def emit_v(kc):
                v_ps = psFill.tile([P, D], f32, name="v_ps", tag="fill")
                for cp in range(2):
                    nc.tensor.matmul(
                        v_ps,
                        lhsT=x8b_sb[:, cp, :, ts(kc, P)],
                        rhs=wv8_sb[:, cp, :, :],
                        start=(cp == 0), stop=(cp == 1),
                        perf_mode=DR, skip_group_check=True,
                    )
                nc.vector.tensor_tensor(
                    V8_sb[:, kc // 2, kc % 2, :, 0:HD],
                    v_ps.rearrange("p (h e) -> p h e", e=HD),
                    bvb_h, op=ALU.add,
                )

            # fill schedule: (h, kc) -> list of closures
            fill = {}
            fill[(0, 1)] = [lambda: emit_k(0, 1)]
            fill[(0, 5)] = [lambda: emit_k(0, 2)]
            fill[(0, 9)] = [lambda: emit_k(0, 3)]
            fill[(0, 13)] = [lambda: emit_q(1, 0)]
            fill[(0, 14)] = [lambda: emit_q(1, 1)]
            fill[(1, 1)] = [lambda: emit_k(1, 0)]
            fill[(1, 3)] = [lambda: emit_k(1, 1)]
            fill[(1, 5)] = [lambda: emit_k(1, 2)]
            fill[(1, 7)] = [lambda: emit_k(1, 3)]
            fill[(1, 9)] = [lambda: emit_q(2, 0)]
            fill[(1, 11)] = [lambda: emit_q(2, 1)]
            fill[(2, 3)] = [lambda: emit_k(2, 0)]
            fill[(2, 7)] = [lambda: emit_k(2, 1)]
            fill[(2, 11)] = [lambda: emit_k(2, 2)]
            fill[(2, 15)] = [lambda: emit_k(2, 3)]
            fill[(3, 5)] = [lambda: emit_q(3, 0)]
            fill[(3, 9)] = [lambda: emit_q(3, 1)]
            fill[(4, 3)] = [lambda: emit_k(3, 0)]
            fill[(4, 7)] = [lambda: emit_k(3, 1)]
            fill[(4, 11)] = [lambda: emit_k(3, 2)]
            fill[(4, 15)] = [lambda: emit_k(3, 3)]

            # upfront projections for head 0
            emit_q(0, 0)
            emit_q(0, 1)
            emit_k(0, 0)

            for h in range(H):
                j, hb = h // 2, 64 * (h % 2)
                ctx_ps = psCtx.tile([VW, TOK], f32, name="ctx_ps", tag="ctx")
                exps = []

                def emit_av(kcp_, e, h=h, ctx_ps=ctx_ps):
                    for th in range(2):
                        nc.tensor.matmul(
                            ctx_ps[:, ts(th, 512)],
                            lhsT=V8_sb[:, kcp_, :, h, :],
                            rhs=e[:, :, ts(th, 512)],
                            start=(kcp_ == 0), stop=(kcp_ == KCP - 1),
                            perf_mode=DR, skip_group_check=True,
                        )

                exp_t = None
                for kc in range(SC):
                    for f_ in fill.get((h, kc), ()):
                        f_()
                    if h == 0 and kc < 14:
                        emit_v(kc)
                    if kc % 2 == 0:
                        exp_t = expP.tile([P, 2, TOK], f8, name="exp8",
                                          tag="exp8")
                        exps.append(exp_t)
                    sc_ps = psSc.tile([P, TOK], f32, name="sc_ps", tag="sc")
                    for th in range(2):
                        nc.tensor.matmul(
                            sc_ps[:, ts(th, 512)],
                            lhsT=K8_sb[hb : hb + HD, :, j, ts(kc, P)],
                            rhs=Q8_sb[hb : hb + HD, :, j, ts(th, 512)],
                            start=True, stop=True,
                            perf_mode=DR, skip_group_check=True,
                        )
                    nc.scalar.activation(exp_t[:, kc % 2, :], sc_ps, AF.Exp,
                                         scale=0.125)
                    if kc % 2 == 1 and kc >= 3:
                        emit_av(kc // 2 - 1, exps[kc // 2 - 1])
                if h == 0:
                    emit_v(14)
                    emit_v(15)
                emit_av(KCP - 1, exps[KCP - 1])

                # normalize off-psum: copy out, recip row 64, bcast, scale
                ctmp = nrmP.tile([VW, TOK], f32, name="ctmp", tag="ctmp")
                nc.vector.tensor_copy(ctmp, ctx_ps)
                rden = nrmP.tile([1, TOK], f32, name="rden", tag="rden")
                nc.vector.reciprocal(rden, ctmp[HD : HD + 1, :])
                rb = nrmP.tile([HD, TOK], f32, name="rb", tag="rb")
                nc.gpsimd.partition_broadcast(rb, rden)
                nc.vector.tensor_tensor(
                    ctxT8_sb[hb : hb + HD, h // 4, (h // 2) % 2, :],
                    ctmp[0:HD, :], rb, op=ALU.mult,
                )

        # ---- post phase: Wo + LN1 + FFN1 (fp8 DR) + FFN2 (bf16) + LN2 ----
        postP = top.enter_context(tc.tile_pool(name="postP", bufs=1))
        ln18_sb = postP.tile([P, 2, 2, TOK], f8)
        ln1g_sb = postP.tile([P, DC, TOK], bf16)
        hid_sb = postP.tile([P, FC, TOK], bf16)
        out_sb = postP.tile([P, DC, TOK], bf16)
        workP = top.enter_context(tc.tile_pool(name="workP", bufs=2))
        ln1t = _ln_alloc(nc, postP, "ln1")
        ln2t = _ln_alloc(nc, postP, "ln2")

        def emit_stats(src_slice, sum_ps, sq_ps, s5, first, last, tagp):
            sbf = workP.tile([P, 512], bf16, name=f"{tagp}_sbf",
                             tag=f"{tagp}_sbf")
            nc.gpsimd.tensor_copy(sbf, src_slice)
            sq = workP.tile([P, 512], bf16, name=f"{tagp}_sq", tag=f"{tagp}_sq")
            nc.vector.tensor_mul(sq, sbf, sbf)
            nc.tensor.matmul(sum_ps[:, s5], lhsT=ones128, rhs=sbf,
                             start=first, stop=last, skip_group_check=True)
            nc.tensor.matmul(sq_ps[:, s5], lhsT=ones128, rhs=sq,
                             start=first, stop=last, skip_group_check=True)

        with tc.tile_pool(name="wo_ps", bufs=3, space="PSUM") as psWo, \
             tc.tile_pool(name="ln1s_ps", bufs=1, space="PSUM") as psS1:
            sum1_ps = psS1.tile([1, TOK], f32, name="ln1_sum")
            sq1_ps = psS1.tile([1, TOK], f32, name="ln1_sqsum")
            for hf in range(2):
                s5 = ts(hf, 512)
                for m in range(DC):
                    wo_ps = psWo.tile([P, 512], f32, name="wo_ps", tag="wo")
                    for cp in range(2):
                        nc.tensor.matmul(
                            wo_ps,
                            lhsT=wo8_sb[:, cp, :, ts(m, P)],
                            rhs=ctxT8_sb[:, cp, :, s5],
                            start=(cp == 0), stop=(cp == 1),
                            perf_mode=DR, skip_group_check=True,
                        )
                    nc.vector.scalar_tensor_tensor(
                        out=spine_sb[:, m, s5], in0=wo_ps,
                        scalar=boT_sb[:, m : m + 1], in1=xq_sb[:, m, s5],
                        op0=ALU.add, op1=ALU.add,
                    )
                    emit_stats(spine_sb[:, m, s5], sum1_ps, sq1_ps, s5,
                               m == 0, m == DC - 1, "s1")
                segs = [(hf * 512, 256), (hf * 512 + 256, 256)]
                _ln_chain(nc, ln1t, sum1_ps, sq1_ps, eps_sb, segs)
                # combine: ln18 (f8, FFN1 input) + ln1g (bf16, FFN2 residual)
                for c in range(DC):
                    v = workP.tile([P, 512], bf16, name="ln1v", tag="ln1v")
                    nc.vector.tensor_mul(v, spine_sb[:, c, s5],
                                         ln1t["bcA"][:, s5])
                    t = workP.tile([P, 512], bf16, name="ln1t", tag="ln1t")
                    nc.vector.tensor_tensor(t, v, ln1t["bcB"][:, s5],
                                            op=ALU.add)
                    nc.gpsimd.tensor_copy(ln18_sb[:, c // 2, c % 2, s5], t)
                    nc.vector.tensor_scalar_mul(
                        ln1g_sb[:, c, s5], t, g1T_sb[:, c : c + 1])

        # FFN1 (fp8 DR) + relu
        with tc.tile_pool(name="f1_ps", bufs=3, space="PSUM") as psF1:
            for hf in range(2):
                s5 = ts(hf, 512)
                for m in range(FC):
                    h_ps = psF1.tile([P, 512], f32, name="h_ps", tag="h")
                    for cp in range(2):
                        nc.tensor.matmul(
                            h_ps,
                            lhsT=w18_sb[:, cp, :, ts(m, P)],
                            rhs=ln18_sb[:, cp, :, s5],
                            start=(cp == 0), stop=(cp == 1),
                            perf_mode=DR, skip_group_check=True,
                        )
                    if m % 2 == 0:
                        nc.scalar.activation(
                            hid_sb[:, m, s5], h_ps, AF.Relu,
                            bias=b1pT_sb[:, m : m + 1], scale=1.0,
                        )
                    else:
                        nc.vector.tensor_scalar(
                            hid_sb[:, m, s5], h_ps, b1pT_sb[:, m : m + 1],
                            0.0, ALU.add, ALU.max)

        # FFN2 (bf16) + LN2, pipelined per token half
        with tc.tile_pool(name="f2_ps", bufs=3, space="PSUM") as psF2, \
             tc.tile_pool(name="ln2s_ps", bufs=1, space="PSUM") as psS2:
            sum2_ps = psS2.tile([1, TOK], f32, name="ln2_sum")
            sq2_ps = psS2.tile([1, TOK], f32, name="ln2_sqsum")
            for hf in range(2):
                s5 = ts(hf, 512)
                for m in range(DC):
                    f_ps = psF2.tile([P, 512], f32, name="f_ps", tag="f")
                    for c in range(FC):
                        nc.tensor.matmul(
                            f_ps,
                            lhsT=w2_sb[:, c, ts(m, P)],
                            rhs=hid_sb[:, c, s5],
                            start=(c == 0), stop=(c == FC - 1),
                            skip_group_check=True,
                        )
                    nc.vector.scalar_tensor_tensor(
                        out=spine_sb[:, m, s5], in0=f_ps,
                        scalar=b2pT_sb[:, m : m + 1], in1=ln1g_sb[:, m, s5],
                        op0=ALU.add, op1=ALU.add,
                    )
                    emit_stats(spine_sb[:, m, s5], sum2_ps, sq2_ps, s5,
                               m == 0, m == DC - 1, "s2")
                segs = [(hf * 512, 256), (hf * 512 + 256, 256)]
                _ln_chain(nc, ln2t, sum2_ps, sq2_ps, eps_sb, segs)
                for c in range(DC):
                    v = workP.tile([P, 512], bf16, name="ln2v", tag="ln2v")
                    nc.vector.tensor_mul(v, spine_sb[:, c, s5],
                                         ln2t["bcA"][:, s5])
                    t = workP.tile([P, 512], bf16, name="ln2t", tag="ln2t")
                    nc.vector.tensor_tensor(t, v, ln2t["bcB"][:, s5],
                                            op=ALU.add)
                    eng = nc.vector if c % 2 == 0 else nc.gpsimd
                    eng.tensor_scalar(
                        out_sb[:, c, s5], t, g2T_sb[:, c : c + 1],
                        beta2T_sb[:, c : c + 1], ALU.mult, ALU.add)
                    nc.sync.dma_start(out=outT_d[:, c, s5],
                                      in_=out_sb[:, c, s5])

    if not nc.is_finalized():
        nc.finalize()
    return nc


def _prep_inputs(x, Wqkv, bqkv, Wo, bo, g1, beta1, W1, b1, W2, b2, g2, beta2):
    """Host-side sharding/layout prep -> list of 8 in_maps."""
    f = lambda a: np.ascontiguousarray(np.asarray(a, dtype=np.float32))
    bf = lambda a: np.ascontiguousarray(
        np.asarray(a, dtype=np.float32).astype(ml_dtypes.bfloat16))
    q8 = lambda a: np.ascontiguousarray(
        np.asarray(a, dtype=np.float32).astype(F8NP))

    def pack8(w):  # [512, N] -> [128, 2, 2, N] fp8, c = 2*cp + i
        w = np.asarray(w, dtype=np.float32)
        return q8(w.reshape(2, 2, P, w.shape[1]).transpose(2, 0, 1, 3))

    def chunkT(w, nchunk, cast):  # [n*128, cols] -> [128, n, cols]
        w = np.asarray(w, dtype=np.float32)
        return cast(w.reshape(nchunk, P, w.shape[1]).transpose(1, 0, 2))

    Wqkv = np.asarray(Wqkv, np.float32)
    s2 = 1.0 / np.sqrt(2.0)
    bqkv_s = np.asarray(bqkv, np.float32).copy()
    bqkv_s[: 2 * D] *= s2                      # q,k bias pre-scaled
    b1p = np.asarray(b1, np.float32) + np.asarray(beta1, np.float32) @ np.asarray(W1, np.float32)
    b2p = np.asarray(b2, np.float32) + np.asarray(beta1, np.float32)
    shared = {
        "wq8": pack8(Wqkv[:, 0:D] * s2),
        "wk8": pack8(Wqkv[:, D : 2 * D] * s2),
        "wv8": pack8(Wqkv[:, 2 * D :]),
        "wo8": pack8(Wo),
        "w18": pack8(np.asarray(W1, np.float32)
                     * np.asarray(g1, np.float32)[:, None] * 16.0),
        "w2": chunkT(np.asarray(W2, np.float32) / 16.0, FC, bf),
        "bqkvT": f(bqkv_s.reshape(12, P).T),
        "boT": f(np.asarray(bo).reshape(DC, P).T),
        "b1pT": f(b1p.reshape(FC, P).T * 16.0),
        "b2pT": f(b2p.reshape(DC, P).T),
        "bvrow": f(np.asarray(bqkv, np.float32)[2 * D :].reshape(1, D)),
        "ones_col": np.ones((P, 1), ml_dtypes.bfloat16),
        "g1T": f(np.asarray(g1).reshape(DC, P).T),
        "g2T": f(np.asarray(g2).reshape(DC, P).T),
        "beta2T": f(np.asarray(beta2).reshape(DC, P).T),
    }
    x = np.asarray(x, dtype=np.float32)
    in_maps = []
    for c in range(8):
        b, half = c // 2, c % 2
        xbT = x[b].T.reshape(2, 2, P, S).transpose(2, 0, 1, 3)   # [128,2,2,S]
        xq = x[b, half * TOK : (half + 1) * TOK]
        xqT4 = xq.T.reshape(DC, P, TOK).transpose(1, 0, 2)        # [128,4,TOK]
        x8qT = xq.T.reshape(2, 2, P, TOK).transpose(2, 0, 1, 3)
        in_maps.append(dict(
            shared, x8b=q8(xbT), x8q=q8(x8qT), xq=bf(xqT4)))
    return in_maps


def kernel(**inputs):
    from concourse.bass_utils import run_bass_kernel_spmd

    nc = _build_program()
    in_maps = _prep_inputs(**inputs)
    res = run_bass_kernel_spmd(nc, in_maps, core_ids=list(range(8)))
    out = np.empty((B, S, D), dtype=np.float32)
    for c in range(8):
        b, half = c // 2, c % 2
        oT = np.asarray(res.results[c]["outT"], dtype=np.float32)  # [P,DC,TOK]
        out[b, half * TOK : (half + 1) * TOK] = (
            oT.transpose(2, 1, 0).reshape(TOK, D)
        )
    return out
